# revision 11
# baseline (speedup 1.0000x reference)
"""Trainium2 Bass kernel for the CapsuleNetwork (MIND-style interest routing).

Reference computation (per batch element b):
    hat = his @ W + bias            # [S, K*D] -> [K, S, D]
    cw = 0
    for i in 0..2:
        sw = softmax_k(cw); sw[mask==0] = 0
        interest = squash(sum_s sw * hat)
        if i < 2: cw += hat . interest
    atten-argmax readout over interests.

Key algebraic restructuring used here (bias is zero per the problem spec):
  hat[b,k,s,:] = his[b,s,:] @ W_k, so by linearity the full `hat` tensor is
  never materialized:
    P[b,k,:]  = sum_s sw[b,k,s]*his[b,s,:]          (per-b PE matmuls)
    v[b,k,:]  = P[b,k,:] @ W_k                      (shared-W_k PE matmuls)
    interest  = c(|v|^2) * v                        (squash, DVE/ACT)
    u[b,k,:]  = W_k @ interest[b,k,:]               (shared-W_k^T PE matmuls)
    cw_i      = his_b @ U_i, U_i = sum_{j<i} u_j    (per-b PE matmuls)
  Mask folding: zeroing sw at masked s is equivalent to folding the mask into
  sw (softmax denominator is over k, not s); cw values at masked s never
  influence the result.
  argmax(softmax(x)) == argmax(x), so the readout softmax is skipped.

Layouts: `his` lives in SBUF twice per 128-batch chunk:
  A-tiles  [s-part, (b, d)]  - stationary for the P matmuls (contraction s)
  HT-tiles [(b-pair, d)-part, s] - stationary for the cw matmuls (contraction d),
                                   built on-chip via PE transposes.
The softmax stack runs in [s-part, (b,k)] layout; squash/readout in
[b-part, (k,d)] layout (also the output layout).

Sharding: pure data parallelism, batch 2048 -> 8 cores x 256.
"""

import sys
import types

sys.path.insert(0, "/opt/trn_rl_repo")

# The agent image's `antenv` package lacks the `axon_hooks` submodule, which
# makes the axon boot silently skip NTFF-profile-hook registration (and with
# it `trace=True` HW timing). Provide the tiny registry it expects before
# anything imports jax/axon.
if "antenv.axon_hooks" not in sys.modules:
    _hooks_mod = types.ModuleType("antenv.axon_hooks")
    _hooks_mod._hook = None

    def _set_hook(h, _m=_hooks_mod):
        _m._hook = h

    def _get_hook(_m=_hooks_mod):
        return _m._hook

    _hooks_mod.set_axon_ntff_profile_hook = _set_hook
    _hooks_mod.get_axon_ntff_profile_hook = _get_hook
    sys.modules["antenv.axon_hooks"] = _hooks_mod
    try:
        import antenv

        antenv.axon_hooks = _hooks_mod
    except ImportError:
        pass
    # If the axon boot already ran (jax imported before us), register the
    # NTFF hook ourselves using the boot module's ctypes shim.
    try:
        if "/root/.axon_site" not in sys.path:
            sys.path.insert(0, "/root/.axon_site")
        from trn_agent_boot.trn_boot import _ntff_profile_via_ctypes

        _h = _ntff_profile_via_ctypes("/opt/axon/libaxon_pjrt.so")
        if _h is not None:
            _set_hook(_h)
    except Exception:
        pass

import numpy as np

import concourse.bass as bass
import concourse.bacc as bacc
import concourse.tile as tile
from concourse import mybir
from concourse.bass_utils import run_bass_kernel_spmd

F32 = mybir.dt.float32
AF = mybir.ActivationFunctionType
ALU = mybir.AluOpType

B, S, D, K = 2048, 200, 64, 4
NCORES = 8
BSH = B // NCORES        # 256 batch rows per core
BC = 128                 # chunk of batch rows processed at once
NCHUNK = BSH // BC       # 2
S1, S2 = 128, S - 128    # s split for 128-partition tiles (128 + 72)


def build_kernel():
    nc = bacc.Bacc()

    his_d = nc.declare_dram_parameter("his", [BSH, S, D], F32, isOutput=False)
    eb_d = nc.declare_dram_parameter("eb", [BSH, D], F32, isOutput=False)
    maskf_d = nc.declare_dram_parameter("maskf", [BSH, S], F32, isOutput=False)
    w_d = nc.declare_dram_parameter("w", [D, K * D], F32, isOutput=False)
    wt_d = nc.declare_dram_parameter("wt", [D, K * D], F32, isOutput=False)
    id_d = nc.declare_dram_parameter("ident", [128, 128], F32, isOutput=False)
    int_d = nc.declare_dram_parameter("interest_out", [BSH, K, D], F32, isOutput=True)
    ro_d = nc.declare_dram_parameter("readout_out", [BSH, D], F32, isOutput=True)

    with tile.TileContext(nc) as tc:
        with (
            tc.tile_pool(name="const", bufs=1) as cpool,
            tc.tile_pool(name="big", bufs=1) as big,
            tc.tile_pool(name="ht", bufs=1) as htp,
            tc.tile_pool(name="sb", bufs=2) as sb,
            tc.tile_pool(name="psA", bufs=2, space=bass.MemorySpace.PSUM) as psA,
            tc.tile_pool(name="psB", bufs=1, space=bass.MemorySpace.PSUM) as psB,
        ):
            w_sb = cpool.tile([D, K * D], F32)
            wt_sb = cpool.tile([D, K * D], F32)
            ident = cpool.tile([128, 128], F32)
            nc.sync.dma_start(w_sb[:], w_d[:])
            nc.sync.dma_start(wt_sb[:], wt_d[:])
            nc.sync.dma_start(ident[:], id_d[:])
            eps = cpool.tile([128, 1], F32)
            nc.gpsimd.memset(eps[:], 1e-9)

            for c in range(NCHUNK):
                bsl = c * BC

                # ---- load his chunk in [s, b, d] layout (two s-tiles) ----
                his_ap = his_d[bsl : bsl + BC].transpose([1, 0, 2])  # [S, BC, D]
                A1 = big.tile([S1, BC, D], F32, tag="A1")
                A2 = big.tile([S2, BC, D], F32, tag="A2")
                nc.sync.dma_start(A1[:], his_ap[0:S1])
                nc.sync.dma_start(A2[:], his_ap[S1:S])

                # ---- mask chunk -> [s, b] transposed tiles ----
                MK = sb.tile([BC, S], F32, tag="MK")
                nc.sync.dma_start(MK[:], maskf_d[bsl : bsl + BC])
                EBt = sb.tile([BC, D], F32, tag="EB")
                nc.sync.dma_start(EBt[:], eb_d[bsl : bsl + BC])

                tpm = psA.tile([128, 128], F32, tag="tp")
                nc.tensor.transpose(tpm[:], MK[:, 0:S1], ident[:])
                MT1 = sb.tile([S1, BC], F32, tag="MT1")
                nc.vector.tensor_copy(MT1[:], tpm[:])
                tpm2 = psA.tile([128, 128], F32, tag="tp")
                nc.tensor.transpose(tpm2[:S2, :], MK[:, S1:S], ident[:])
                MT2 = sb.tile([S2, BC], F32, tag="MT2")
                nc.vector.tensor_copy(MT2[:], tpm2[:S2, :])

                # ---- hisT via PE transposes: HT[(pair-half,d), j, s] ----
                HT1 = htp.tile([128, BC // 2, S1], F32, tag="HT1")
                HT2 = htp.tile([128, BC // 2, S2], F32, tag="HT2")
                for g in range(BC // 8):  # groups of 4 pairs = 8 batch rows
                    tp4 = psA.tile([128, 512], F32, tag="tp")
                    for q in range(4):
                        j = 4 * g + q
                        nc.tensor.transpose(
                            tp4[:, q * 128 : q * 128 + 128],
                            A1[:, 2 * j : 2 * j + 2, :],
                            ident[:],
                        )
                    cp = nc.vector.tensor_copy if g % 2 == 0 else nc.scalar.copy
                    cp(
                        HT1[:, 4 * g : 4 * g + 4, :],
                        tp4[:].rearrange("p (q s) -> p q s", q=4),
                    )
                    tp4b = psA.tile([128, 512], F32, tag="tp")
                    for q in range(4):
                        j = 4 * g + q
                        nc.tensor.transpose(
                            tp4b[:, q * 128 : q * 128 + S2],
                            A2[:, 2 * j : 2 * j + 2, :],
                            ident[:S2, :S2],
                        )
                    cp(
                        HT2[:, 4 * g : 4 * g + 4, :],
                        tp4b[:].rearrange("p (q s) -> p q s", q=4)[:, :, 0:S2],
                    )

                # iter-0 routing weights: sw = 0.25 * mask (same for all k)
                SM1_0 = sb.tile([S1, BC, K], F32, tag="SM1")
                SM2_0 = sb.tile([S2, BC, K], F32, tag="SM2")
                nc.scalar.mul(
                    SM1_0[:], MT1[:].unsqueeze(2).broadcast_to((S1, BC, K)), 0.25
                )
                nc.scalar.mul(
                    SM2_0[:], MT2[:].unsqueeze(2).broadcast_to((S2, BC, K)), 0.25
                )

                U_prev = None  # SBUF [D, K, BC] accumulated u (k-major)
                SM1, SM2 = SM1_0, SM2_0
                for it in range(3):
                    if it > 0:
                        # ---- cw = his_b @ U  -> PSUM [s, (b,k)] ----
                        CW1 = psB.tile([S1, BC, K], F32, tag="CW1")
                        CW2 = psB.tile([S2, BC, K], F32, tag="CW2")
                        for b in range(BC):
                            j, h = b // 2, b % 2
                            rhs = U_prev[h * 64 : h * 64 + 64].transpose([0, 2, 1])[
                                :, b, :
                            ]
                            nc.tensor.matmul(
                                CW1[:, b, :], HT1[h * 64 : h * 64 + 64, j, :], rhs
                            )
                            nc.tensor.matmul(
                                CW2[:, b, :], HT2[h * 64 : h * 64 + 64, j, :], rhs
                            )
                        # ---- softmax over k + mask fold, in [s,(b,k)] ----
                        E1 = sb.tile([S1, BC, K], F32, tag="E1")
                        E2 = sb.tile([S2, BC, K], F32, tag="E2")
                        nc.scalar.activation(E1[:], CW1[:], AF.Exp)
                        nc.scalar.activation(E2[:], CW2[:], AF.Exp)
                        DN1 = sb.tile([S1, BC], F32, tag="DN1")
                        DN2 = sb.tile([S2, BC], F32, tag="DN2")
                        nc.vector.reduce_sum(DN1[:], E1[:], axis=mybir.AxisListType.X)
                        nc.vector.reduce_sum(DN2[:], E2[:], axis=mybir.AxisListType.X)
                        RC1 = sb.tile([S1, BC], F32, tag="RC1")
                        RC2 = sb.tile([S2, BC], F32, tag="RC2")
                        nc.vector.reciprocal(RC1[:], DN1[:])
                        nc.vector.reciprocal(RC2[:], DN2[:])
                        RM1 = sb.tile([S1, BC], F32, tag="RM1")
                        RM2 = sb.tile([S2, BC], F32, tag="RM2")
                        nc.vector.tensor_mul(RM1[:], RC1[:], MT1[:])
                        nc.vector.tensor_mul(RM2[:], RC2[:], MT2[:])
                        SM1 = sb.tile([S1, BC, K], F32, tag="SM1")
                        SM2 = sb.tile([S2, BC, K], F32, tag="SM2")
                        nc.vector.tensor_mul(
                            SM1[:], E1[:], RM1[:].unsqueeze(2).broadcast_to((S1, BC, K))
                        )
                        nc.vector.tensor_mul(
                            SM2[:], E2[:], RM2[:].unsqueeze(2).broadcast_to((S2, BC, K))
                        )

                    # ---- P^T = sum_s sw * his : per-b matmuls -> PSUM [d,(b,k)] ----
                    PT = psB.tile([D, BC, K], F32, tag="PT")
                    for b in range(BC):
                        nc.tensor.matmul(
                            PT[:, b, :], A1[:, b, :], SM1[:, b, :],
                            start=True, stop=False,
                        )
                        nc.tensor.matmul(
                            PT[:, b, :], A2[:, b, :], SM2[:, b, :],
                            start=False, stop=True,
                        )
                    PS = sb.tile([D, BC, K], F32, tag="PS")
                    nc.vector.tensor_copy(PS[:], PT[:])

                    # ---- v^T = W_k @ P^T (k-major) ----
                    VT = psB.tile([D, K, BC], F32, tag="VT")
                    for k in range(K):
                        nc.tensor.matmul(
                            VT[:, k, :],
                            w_sb[:, k * D : (k + 1) * D],
                            PS[:].transpose([0, 2, 1])[:, k, :],
                        )
                    VS = sb.tile([D, K, BC], F32, tag="VS")
                    nc.vector.tensor_copy(VS[:], VT[:])

                    # ---- transpose v -> [b, (k,d)] ----
                    VB = psB.tile([BC, K, D], F32, tag="VB")
                    for k in range(K):
                        nc.tensor.transpose(VB[:, k, :], VS[:, k, :], ident[:D, :D])

                    # ---- squash ----
                    SQ = sb.tile([BC, K * D], F32, tag="SQ")
                    nc.scalar.activation(SQ[:], VB[:].rearrange("p k d -> p (k d)"), AF.Square)
                    N2 = sb.tile([BC, K], F32, tag="N2")
                    nc.vector.reduce_sum(
                        N2[:], SQ[:].rearrange("p (k d) -> p k d", k=K),
                        axis=mybir.AxisListType.X,
                    )
                    SQR = sb.tile([BC, K], F32, tag="SQR")
                    nc.scalar.activation(SQR[:], N2[:], AF.Sqrt, bias=eps[:BC, :])
                    T1 = sb.tile([BC, K], F32, tag="T1")
                    nc.scalar.add(T1[:], N2[:], 1.0)
                    DEN = sb.tile([BC, K], F32, tag="DEN")
                    nc.vector.tensor_mul(DEN[:], T1[:], SQR[:])
                    RD = sb.tile([BC, K], F32, tag="RD")
                    nc.vector.reciprocal(RD[:], DEN[:])
                    CC = sb.tile([BC, K], F32, tag="CC")
                    nc.vector.tensor_mul(CC[:], N2[:], RD[:])
                    IS = sb.tile([BC, K, D], F32, tag="IS")
                    nc.vector.tensor_mul(
                        IS[:], VB[:], CC[:].unsqueeze(2).broadcast_to((BC, K, D))
                    )

                    if it < 2:
                        # ---- interest^T via per-k PE transposes (base-0 tiles) ----
                        tpi = psA.tile([D, K, BC], F32, tag="tp")
                        for k in range(K):
                            nc.tensor.transpose(tpi[:, k, :], IS[:, k, :], ident[:])
                        IT = sb.tile([D, K, BC], F32, tag="IT")
                        nc.scalar.copy(IT[:], tpi[:])
                        # ---- u^T = W_k @ interest^T, accumulate U ----
                        UP = psB.tile([D, K, BC], F32, tag="UP")
                        for k in range(K):
                            nc.tensor.matmul(
                                UP[:, k, :],
                                wt_sb[:, k * D : (k + 1) * D],
                                IT[:, k, :],
                            )
                        # U lives duplicated on partitions 0-63 and 64-127 so the
                        # cw matmuls (stationary at base 0 or 64) see a matching
                        # base partition.
                        U_new = sb.tile([128, K, BC], F32, tag=f"U{it}")
                        if U_prev is None:
                            nc.vector.tensor_copy(U_new[0:D], UP[:])
                        else:
                            nc.vector.tensor_add(U_new[0:D], UP[:], U_prev[0:D])
                        nc.sync.dma_start(U_new[D : 2 * D], U_new[0:D])
                        U_prev = U_new
                    else:
                        # ---- outputs: interest + argmax readout ----
                        nc.sync.dma_start(int_d[bsl : bsl + BC], IS[:])
                        LT = sb.tile([BC, K, D], F32, tag="LT")
                        nc.vector.tensor_mul(
                            LT[:], IS[:],
                            EBt[:].unsqueeze(1).broadcast_to((BC, K, D)),
                        )
                        LG = sb.tile([BC, K], F32, tag="LG")
                        nc.vector.reduce_sum(LG[:], LT[:], axis=mybir.AxisListType.X)
                        MX = sb.tile([BC, 1], F32, tag="MX")
                        nc.vector.reduce_max(MX[:], LG[:], axis=mybir.AxisListType.X)
                        EQ = sb.tile([BC, K], F32, tag="EQ")
                        nc.vector.tensor_tensor(
                            EQ[:], LG[:], MX[:].broadcast_to((BC, K)),
                            op=ALU.is_equal,
                        )
                        # first-match one-hot (ties -> lowest k, like jnp.argmax)
                        FH = sb.tile([BC, K], F32, tag="FH")
                        CUM = sb.tile([BC, 3], F32, tag="CUM")
                        nc.vector.tensor_copy(FH[:, 0:1], EQ[:, 0:1])
                        nc.scalar.activation(
                            CUM[:, 0:1], EQ[:, 0:1], AF.Copy, scale=-1.0
                        )
                        nc.scalar.add(CUM[:, 0:1], CUM[:, 0:1], 1.0)
                        for k in range(1, K):
                            nc.vector.tensor_mul(
                                FH[:, k : k + 1], EQ[:, k : k + 1],
                                CUM[:, k - 1 : k],
                            )
                            if k < K - 1:
                                nc.scalar.activation(
                                    CUM[:, k : k + 1], EQ[:, k : k + 1],
                                    AF.Copy, scale=-1.0,
                                )
                                nc.scalar.add(CUM[:, k : k + 1], CUM[:, k : k + 1], 1.0)
                                nc.vector.tensor_mul(
                                    CUM[:, k : k + 1], CUM[:, k : k + 1],
                                    CUM[:, k - 1 : k],
                                )
                        RT = sb.tile([BC, K, D], F32, tag="RT")
                        nc.vector.tensor_mul(
                            RT[:], IS[:],
                            FH[:].unsqueeze(2).broadcast_to((BC, K, D)),
                        )
                        RO = sb.tile([BC, D], F32, tag="RO")
                        nc.vector.reduce_sum(
                            RO[:], RT[:].transpose([0, 2, 1]),
                            axis=mybir.AxisListType.X,
                        )
                        nc.sync.dma_start(ro_d[bsl : bsl + BC], RO[:])
    nc.compile()
    return nc


_NC = None


def _get_nc():
    global _NC
    if _NC is None:
        _NC = build_kernel()
    return _NC


def kernel(item_his_emb, item_eb, mask, W, b, _trace=False):
    nc = _get_nc()
    Wf = np.asarray(W, dtype=np.float32)
    WT = np.concatenate(
        [np.ascontiguousarray(Wf[:, k * D : (k + 1) * D].T) for k in range(K)], axis=1
    )
    ident = np.eye(128, dtype=np.float32)
    maskf = np.asarray(mask).astype(np.float32)
    his = np.asarray(item_his_emb, dtype=np.float32)
    eb = np.asarray(item_eb, dtype=np.float32)

    in_maps = []
    for i in range(NCORES):
        sl = slice(i * BSH, (i + 1) * BSH)
        in_maps.append(
            {
                "his": np.ascontiguousarray(his[sl]),
                "eb": np.ascontiguousarray(eb[sl]),
                "maskf": np.ascontiguousarray(maskf[sl]),
                "w": Wf,
                "wt": WT,
                "ident": ident,
            }
        )
    res = run_bass_kernel_spmd(
        nc, in_maps, core_ids=list(range(NCORES)), trace=_trace
    )
    interest = np.concatenate([r["interest_out"] for r in res.results], axis=0)
    readout = np.concatenate([r["readout_out"] for r in res.results], axis=0)
    if _trace:
        kernel.last_exec_time_ns = res.exec_time_ns
        kernel.last_trace = res.instructions_and_trace
    return interest, readout
